# revision 40
# baseline (speedup 1.0000x reference)
"""Trainium2 Bass kernel for nn_MultiHeadAttention (RoPE MHA, B=2 S=2048 E=1024 H=16).

Sharding: tensor-parallel over heads — 2 heads per core on 8 cores. Each core
computes its heads' q/k/v projections, RoPE, attention, and the partial output
projection (its rows of Wo); the host sums the 8 partials and adds bo.

Device layouts: q/k as [d, token] (transposed) so attention scores come out as
[ks, qs]; softmax's row-sum falls out of the same matmul that computes ctx via
a ones column appended to v. rotate_half is a signed-permutation matmul.
v is projected transposed (full-width matmuls) and PE-transposed to natural.

Restructure vs the original baseline (313us -> 243us):
- kt loop software-pipelined: scores(kt) emitted before ctx(kt-1) so the
  tensor engine is not serialized behind ACT's exp each iteration; exp on
  ACT is the steady-state pacer (~1.11us per [128,1024] kt tile).
- PSUM pools separated: scores 2x2 banks, ctx accumulators 2x1, rotating
  work pool 2x1; the upfront (boot) projection chunks borrow the then-idle
  score/ctx pools so the projection prologue pipelines without slot stalls.
- Z normalization with NO DMA chain: Z row copies + fast custom-DVE
  reciprocal_approx_fast on the [1,1024] row + gpsimd partition_broadcast
  to [64,1024], then deferred DVE multiply pieces. (Earlier DRAM-bounce /
  transpose-gather designs serialized every block boundary through in-order
  DMA queues — the single biggest loss in the baseline.)
- Filler scheduler: proj chunks (blocks 0-3) and out-proj tiles (lag >= 2,
  balanced map {4:(0,1), 5:(2,3), 6:(4,5), 7:(6,)}) drain max 2 pieces per
  kt slot, none in the last two slots; epilogue pieces defer into the next
  block so no cross-engine wait ever blocks a queue head.
- Tail: last block's normalize is split per 128-token slice, each slice's
  out-proj follows immediately (ACT does the e=1 evictions, idle post-exp).
- DMA queues: upfront loads split across sync/scalar/gpsimd; steady-state
  xt + all yp on sync; z-path on gpsimd compute; yp written bf16;
  cos/sin loaded once as persistent [128, S] f32 tables.
"""

import os
import sys
from collections import deque
from contextlib import ExitStack

import numpy as np

for _p in ("/opt/trn_rl_repo", "/opt/pypackages"):
    if _p not in sys.path and os.path.isdir(_p):
        sys.path.append(_p)

import concourse.bass as bass
import concourse.mybir as mybir
import concourse.tile as tile
from concourse import bacc
from concourse import bass_utils
from concourse.masks import make_identity

F32 = mybir.dt.float32
BF16 = mybir.dt.bfloat16
AF = mybir.ActivationFunctionType
OP = mybir.AluOpType

B = 2
S = 2048
E = 1024
H = 16
D = 64
N_CORES = 8
HPC = H // N_CORES  # heads per core = 2
HD = HPC * D  # 128

MM_MODE = "bf16"  # matmul input dtype (fixed; kept for the test harness banner)
LAST_RESULTS = None  # BassKernelResults of the most recent run (for test harness)
_NC_CACHE = {}


def build_mha_nc():
    T = B * S
    TC = 512  # token chunk for projections
    NCH = T // TC  # 8 proj chunks
    QC = 512  # query chunk in attention
    NQC = S // QC  # 4 blocks per batch
    NKT = S // 128  # 16 key tiles per batch
    KE = E // 128  # 8 contraction tiles for projections

    dt_in = BF16
    nc = bacc.Bacc(None, target_bir_lowering=False, debug=False)

    xT = nc.dram_tensor("xT", [E, T], dt_in, kind="ExternalInput")
    wq = nc.dram_tensor("wq", [E, HD], dt_in, kind="ExternalInput")
    wk = nc.dram_tensor("wk", [E, HD], dt_in, kind="ExternalInput")
    wv = nc.dram_tensor("wv", [E, HD], dt_in, kind="ExternalInput")
    bq = nc.dram_tensor("bq", [HD, 1], F32, kind="ExternalInput")
    bk = nc.dram_tensor("bk", [HD, 1], F32, kind="ExternalInput")
    bv = nc.dram_tensor("bv", [HD, 1], F32, kind="ExternalInput")
    wo = nc.dram_tensor("wo", [HD, E], dt_in, kind="ExternalInput")
    cosS = nc.dram_tensor("cosS", [HD, S], F32, kind="ExternalInput")
    sinS = nc.dram_tensor("sinS", [HD, S], F32, kind="ExternalInput")
    rot = nc.dram_tensor("rot", [HD, HD], dt_in, kind="ExternalInput")
    yp = nc.dram_tensor("yp", [T, E], BF16, kind="ExternalOutput")

    scale = 1.0 / np.sqrt(D)

    with tile.TileContext(nc) as tc, ExitStack() as ctx:
        const = ctx.enter_context(tc.tile_pool(name="const", bufs=1))
        xt_pool = ctx.enter_context(tc.tile_pool(name="xt", bufs=4 * KE))
        qkraw_pool = ctx.enter_context(tc.tile_pool(name="qkraw", bufs=4))
        rope_tmp = ctx.enter_context(tc.tile_pool(name="ropetmp", bufs=4))
        persist = ctx.enter_context(tc.tile_pool(name="persist", bufs=1))
        exps_pool = ctx.enter_context(tc.tile_pool(name="exps", bufs=8))
        zr_pool = ctx.enter_context(tc.tile_pool(name="zr", bufs=4))
        zc_pool = ctx.enter_context(tc.tile_pool(name="zc", bufs=4))
        zb_pool = ctx.enter_context(tc.tile_pool(name="zb", bufs=3))
        cun_pool = ctx.enter_context(tc.tile_pool(name="cun", bufs=6))
        osb_pool = ctx.enter_context(tc.tile_pool(name="osb", bufs=8))
        dram = ctx.enter_context(tc.tile_pool(name="dram", bufs=8, space="DRAM"))

        # PSUM: scores 2 slots x 2 banks; ctx accum 2 x 1; rotating work 2 x 1
        ps_score = ctx.enter_context(tc.tile_pool(name="ps_score", bufs=2, space="PSUM"))
        ps_ctx = ctx.enter_context(tc.tile_pool(name="ps_ctx", bufs=2, space="PSUM"))
        ps_work = ctx.enter_context(tc.tile_pool(name="ps_work", bufs=2, space="PSUM"))

        # ---- constants to SBUF, interleaved with chunk-0 x tiles on the
        # sync queue so the first projection matmul starts ASAP ----
        wq_sb = [None] * KE
        wk_sb = [None] * KE
        wv_sb = [None] * KE
        xt0 = [None] * KE
        for k in range(KE):
            q1, q2 = (nc.sync, nc.scalar) if k % 2 == 0 else (nc.scalar, nc.sync)
            for i, (nm, dr, arr) in enumerate(
                (("wq", wq, wq_sb), ("wk", wk, wk_sb), ("wv", wv, wv_sb))
            ):
                t = const.tile([128, HD], dt_in, name=f"{nm}_{k}", tag=f"{nm}_{k}")
                (q1 if i % 2 == 0 else q2).dma_start(t[:], dr.ap()[128 * k : 128 * (k + 1), :])
                arr[k] = t
            t = xt_pool.tile([128, TC], dt_in, name=f"xt_0_{k}", tag="xt")
            q2.dma_start(t[:], xT.ap()[128 * k : 128 * (k + 1), 0:TC])
            xt0[k] = t
            if k == 0:
                bq_sb = const.tile([HD, 1], F32, name="bq_sb", tag="bq_sb")
                nc.sync.dma_start(bq_sb[:], bq.ap())
                bk_sb = const.tile([HD, 1], F32, name="bk_sb", tag="bk_sb")
                nc.sync.dma_start(bk_sb[:], bk.ap())
                bv_sb = const.tile([HD, 1], F32, name="bv_sb", tag="bv_sb")
                nc.sync.dma_start(bv_sb[:], bv.ap())
        rot_sb = const.tile([HD, HD], dt_in, name="rot_sb", tag="rot_sb")
        nc.scalar.dma_start(rot_sb[:], rot.ap())
        wo_sb = const.tile([HD, E], dt_in, name="wo_sb", tag="wo_sb")
        nc.gpsimd.dma_start(wo_sb[:], wo.ap())
        # cos/sin persistent; split across scalar/gpsimd (needed by first rope)
        cos_sb = const.tile([HD, S], F32, name="cos_sb", tag="cos_sb")
        nc.scalar.dma_start(cos_sb[:], cosS.ap())
        sin_sb = const.tile([HD, S], F32, name="sin_sb", tag="sin_sb")
        nc.gpsimd.dma_start(sin_sb[:], sinS.ap())
        ident = const.tile([128, 128], dt_in, name="ident", tag="ident")
        make_identity(nc, ident)

        # ---- persistent intermediates ----
        q_rope = persist.tile([HD, T], dt_in, name="q_rope", tag="q_rope")
        k_rope = persist.tile([HD, T], dt_in, name="k_rope", tag="k_rope")
        v_sb = []
        for i in range(T // 128):
            t = persist.tile([128, HPC * (D + 1)], dt_in, name=f"v_{i}", tag=f"v_{i}")
            v_sb.append(t)
        ctx_pack = {}
        for b in range(B):
            ctx_pack[b] = persist.tile([HD, S], dt_in, name=f"ctxp_{b}", tag=f"ctxp_{b}")

        # ---- stage 1: projections (qT/kT/vT) + RoPE + v transpose ----
        # Emitted as a list of closures (pieces) so attention kt-loops can
        # interleave them as tensor-engine filler.
        def proj_chunk_pieces(c, evict_dve, boot=False):
            psq_pool = ps_score if boot else ps_work
            psk_pool = ps_score if boot else ps_work
            psv_pool = ps_ctx if boot else ps_work
            psr_pool = ps_ctx if boot else ps_work
            tq = "ps_score" if boot else "ps_work"
            tv = "ps_ctx" if boot else "ps_work"
            # q/k evictions always on DVE; v on ACT for upfront chunks (ACT
            # idle during proj) and DVE when interleaved with attention.
            c0 = TC * c
            s0 = c0 % S  # cos/sin column offset (tables are per-batch)
            state = {}

            def evict(dst, ps, b_sb, force_dve=True):
                if evict_dve or force_dve:
                    nc.vector.tensor_scalar_add(dst, ps, b_sb[:, 0:1])
                else:
                    nc.scalar.activation(dst, ps, AF.Identity, bias=b_sb[:, 0:1])

            def load_x():
                if c == 0:
                    state["xt"] = xt0
                    return
                xt = [None] * KE
                for k in range(KE):
                    t = xt_pool.tile([128, TC], dt_in, name=f"xt_{c}_{k}", tag="xt")
                    # boot chunks split across sync/scalar; streamed chunks
                    # stay off the scalar queue (ACT is the exp pacer there)
                    if boot and k % 2 == 1:
                        q = nc.scalar
                    elif boot and k % 4 == 2:
                        q = nc.gpsimd
                    else:
                        q = nc.sync
                    q.dma_start(t[:], xT.ap()[128 * k : 128 * (k + 1), c0 : c0 + TC])
                    xt[k] = t
                state["xt"] = xt

            def mm_q():
                ps = psq_pool.tile([HD, TC], F32, name="psq", tag=tq)
                for k in range(KE):
                    nc.tensor.matmul(ps[:], wq_sb[k][:], state["xt"][k][:],
                                     start=(k == 0), stop=(k == KE - 1))
                raw = qkraw_pool.tile([HD, TC], dt_in, name="qraw", tag="qkraw")
                evict(raw[:], ps[:], bq_sb)
                state["qraw"] = raw

            def rope_q():
                raw = state["qraw"]
                psrot = psr_pool.tile([HD, TC], F32, name="psrotq", tag=tv)
                nc.tensor.matmul(psrot[:], rot_sb[:], raw[:], start=True, stop=True)
                sprod = rope_tmp.tile([HD, TC], F32, name="sprodq", tag="ropetmp")
                nc.vector.tensor_tensor(sprod[:], psrot[:], sin_sb[:, s0 : s0 + TC], op=OP.mult)
                cprod = rope_tmp.tile([HD, TC], F32, name="cprodq", tag="ropetmp")
                nc.vector.tensor_tensor(cprod[:], raw[:], cos_sb[:, s0 : s0 + TC], op=OP.mult)
                eng = nc.gpsimd if boot else nc.vector
                eng.tensor_tensor(q_rope[:, c0 : c0 + TC], cprod[:], sprod[:], op=OP.add)

            def mm_k():
                ps = psk_pool.tile([HD, TC], F32, name="psk", tag=tq)
                for k in range(KE):
                    nc.tensor.matmul(ps[:], wk_sb[k][:], state["xt"][k][:],
                                     start=(k == 0), stop=(k == KE - 1))
                raw = qkraw_pool.tile([HD, TC], dt_in, name="kraw", tag="qkraw")
                evict(raw[:], ps[:], bk_sb)
                state["kraw"] = raw

            def rope_k():
                raw = state["kraw"]
                psrot = psr_pool.tile([HD, TC], F32, name="psrotk", tag=tv)
                nc.tensor.matmul(psrot[:], rot_sb[:], raw[:], start=True, stop=True)
                sprod = rope_tmp.tile([HD, TC], F32, name="sprodk", tag="ropetmp")
                nc.vector.tensor_tensor(sprod[:], psrot[:], sin_sb[:, s0 : s0 + TC], op=OP.mult)
                cprod = rope_tmp.tile([HD, TC], F32, name="cprodk", tag="ropetmp")
                nc.vector.tensor_tensor(cprod[:], raw[:], cos_sb[:, s0 : s0 + TC], op=OP.mult)
                eng = nc.gpsimd if boot else nc.vector
                eng.tensor_tensor(k_rope[:, c0 : c0 + TC], cprod[:], sprod[:], op=OP.add)

            def mm_v():
                ps = psv_pool.tile([HD, TC], F32, name="psv", tag=tv)
                for k in range(KE):
                    nc.tensor.matmul(ps[:], wv_sb[k][:], state["xt"][k][:],
                                     start=(k == 0), stop=(k == KE - 1))
                raw = qkraw_pool.tile([HD, TC], dt_in, name="vraw", tag="qkraw")
                evict(raw[:], ps[:], bv_sb, force_dve=False)
                state["vraw"] = raw

            def v_tr():
                vraw = state["vraw"]
                for j in range(TC // 128):
                    pvt = ps_work.tile([128, 128], dt_in, name="pvt", tag="ps_work")
                    nc.tensor.transpose(pvt[:], vraw[:, 128 * j : 128 * (j + 1)], ident[:])
                    vt = v_sb[(c0 + 128 * j) // 128]
                    for h in range(HPC):
                        nc.vector.memset(vt[:, (D + 1) * h + D : (D + 1) * (h + 1)], 1.0)
                        nc.vector.tensor_copy(
                            vt[:, (D + 1) * h : (D + 1) * h + D],
                            pvt[:, D * h : D * (h + 1)],
                        )

            return [load_x, mm_q, mm_k, rope_q, mm_v, rope_k, v_tr]

        # ---- stage 3: output projection for one (batch, 128-token tile) ----
        def outproj_piece(b, j, tail=False):
            def run():
                t0 = b * S
                for e in range(E // 512):
                    pso = ps_work.tile([128, 512], F32, name="pso", tag="ps_work")
                    nc.tensor.matmul(
                        pso[:],
                        ctx_pack[b][:, 128 * j : 128 * (j + 1)],
                        wo_sb[:, 512 * e : 512 * (e + 1)],
                        start=True, stop=True,
                    )
                    osb = osb_pool.tile([128, 512], BF16, name="osb", tag="osb")
                    if tail and e == 1:
                        # ACT is idle once the exps are done; parallel eviction
                        nc.scalar.activation(osb[:], pso[:], AF.Identity)
                    else:
                        nc.vector.tensor_copy(osb[:], pso[:])
                    # yp on sync (fast hw queue); scalar in the tail (ACT idle)
                    eng = nc.scalar if (tail and e == 1) else nc.sync
                    eng.dma_start(
                        yp.ap()[t0 + 128 * j : t0 + 128 * (j + 1), 512 * e : 512 * (e + 1)],
                        osb[:],
                    )
            return run

        # ---- stage 2: attention for one (batch, query-chunk) with fillers ----
        def qc_block(b, qc, fillers, last=False):
            t0 = b * S
            q0 = t0 + QC * qc
            psc = [
                ps_ctx.tile([D + 1, QC], F32, name=f"psctx{h}", tag="ps_ctx")
                for h in range(HPC)
            ]
            ex_tiles = [None] * NKT

            def emit_ctx(kt):
                k0 = t0 + 128 * kt
                for h in range(HPC):
                    nc.tensor.matmul(
                        psc[h][:],
                        v_sb[k0 // 128][:, (D + 1) * h : (D + 1) * (h + 1)],
                        ex_tiles[kt][:, QC * h : QC * (h + 1)],
                        start=(kt == 0), stop=(kt == NKT - 1),
                    )

            # fillers spread over the kt slots (max 2 per slot, +2 slack so
            # the last slots stay clear of filler work); leftovers spill into
            # the next block's slots
            nfill = len(fillers)
            for kt in range(NKT):
                k0 = t0 + 128 * kt
                pss = ps_score.tile([128, HPC * QC], F32, name="pss", tag="ps_score")
                for h in range(HPC):
                    nc.tensor.matmul(
                        pss[:, QC * h : QC * (h + 1)],
                        k_rope[D * h : D * (h + 1), k0 : k0 + 128],
                        q_rope[D * h : D * (h + 1), q0 : q0 + QC],
                        start=True, stop=True,
                    )
                ex = exps_pool.tile([128, HPC * QC], dt_in, name="ex", tag="exps")
                nc.scalar.activation(ex[:], pss[:], AF.Exp, scale=scale)
                ex_tiles[kt] = ex
                if kt >= 1:
                    emit_ctx(kt - 1)
                due = nfill * (kt + 1) // ((NKT - 4) if last else (NKT + 2))
                done = nfill - len(fillers)
                pulls = 0
                while fillers and done + pulls < due and pulls < 2:
                    fillers.popleft()()
                    pulls += 1
            emit_ctx(NKT - 1)
            for p in reversed(finish_block(b, qc, psc, tail_mode=last)):
                fillers.appendleft(p)

        # ---- block epilogue: psc eviction inline (frees PSUM fast); Z is
        # broadcast across partitions with a gpsimd partition_broadcast (no
        # DMA, no DRAM bounce) and the normalize becomes a deferred DVE
        # tensor_tensor divide, so no engine ever waits on a DMA chain ----
        def finish_block(b, qc, psc, tail_mode=False):
            qs = slice(QC * qc, QC * (qc + 1))
            zrow = zr_pool.tile([1, HPC * QC], F32, name="zrow", tag="zrow")
            cun = []
            for h in range(HPC):
                nc.vector.tensor_copy(zrow[:, QC * h : QC * (h + 1)], psc[h][D : D + 1, :])
                cu = cun_pool.tile([D, QC], dt_in, name=f"cun{h}", tag="cun")
                nc.vector.tensor_copy(cu[:], psc[h][0:D, :])
                cun.append(cu)
            zrec = zr_pool.tile([1, HPC * QC], F32, name="zrec", tag="zrec")
            nc.vector.reciprocal_approx_fast(zrec[:], zrow[:])
            zb = zb_pool.tile([D, HPC * QC], F32, name="zb", tag="zb")
            nc.gpsimd.partition_broadcast(zb[:], zrec[:])

            def norm_slice(jlo, jhi):
                js = slice(QC * qc + 128 * jlo, QC * qc + 128 * jhi)
                cs = slice(128 * jlo, 128 * jhi)
                nc.vector.tensor_tensor(
                    ctx_pack[b][0:D, js], cun[0][:, cs], zb[:, cs], op=OP.mult
                )
                csh = cun_pool.tile([D, 128 * (jhi - jlo)], dt_in, name="csh2", tag="csh2")
                nc.vector.tensor_tensor(
                    csh[:], cun[1][:, cs], zb[:, QC + 128 * jlo : QC + 128 * jhi], op=OP.mult
                )
                (nc.scalar if tail_mode else nc.sync).dma_start(
                    ctx_pack[b][D : 2 * D, js], csh[:]
                )

            return [lambda: norm_slice(0, 4)] if not tail_mode else [
                (lambda j=j: norm_slice(j, j + 1)) for j in range(QC // 128)
            ]

        # ---- schedule ----
        # upfront: batch-0 projections (chunks 0-3); ACT does the evictions
        boot_pieces = [
            proj_chunk_pieces(c, evict_dve=False, boot=True) for c in range(NCH // B)
        ]
        for pieces in boot_pieces:  # all x loads first: DMA supply leads compute
            pieces[0]()
        for pieces in boot_pieces:
            for piece in pieces[1:]:
                piece()

        # blocks 0-3 (batch 0): interleave batch-1 proj chunks (DVE evictions)
        # blocks 2-7: interleave out-proj of block idx-2 (lag 2 so ctx_pack
        # and its Z normalization are long finished)
        JT = S // 128  # out-proj token tiles per batch
        def outproj_js(idx):
            b, qc = blocks[idx]
            return [(b, j) for j in range(JT * qc // NQC, JT * (qc + 1) // NQC)]

        fillers = deque()
        blocks = [(b, qc) for b in range(B) for qc in range(NQC)]
        for idx, (b, qc) in enumerate(blocks):
            if idx < NCH - NCH // B:
                for piece in proj_chunk_pieces(NCH // B + idx, evict_dve=True):
                    fillers.append(piece)
            # out-proj filler map balancing per-block load: blocks 0-3 carry
            # the proj chunks; blocks 4-6 carry two blocks' out-proj each;
            # block 7 carries one; the last block's own out-proj is the tail
            opmap = {4: (0, 1), 5: (2, 3), 6: (4, 5), 7: (6,)}
            for oidx in opmap.get(idx, ()):
                for ob, j in outproj_js(oidx):
                    fillers.append(outproj_piece(ob, j))
            qc_block(b, qc, fillers, last=(idx == len(blocks) - 1))
        # tail: norm slice j of the last block, then its out-proj, interleaved
        last_ops = [outproj_piece(ob, j, tail=True) for ob, j in outproj_js(len(blocks) - 1)]
        norm_ops = []
        while fillers and len(norm_ops) < len(last_ops):
            norm_ops.append(fillers.popleft())
        while fillers:
            fillers.popleft()()
        for n_op, o_op in zip(norm_ops, last_ops):
            n_op()
            o_op()

    nc.compile()
    return nc


def _rope_tables():
    inv_freq = 1.0 / (10000.0 ** (np.arange(0, D, 2, dtype=np.float32) / D))
    t = np.arange(S, dtype=np.float32)
    freqs = np.outer(t, inv_freq).astype(np.float32)
    emb = np.concatenate([freqs, freqs], axis=-1)
    return np.cos(emb).astype(np.float32), np.sin(emb).astype(np.float32)


def _rot_matrix():
    R = np.zeros((HD, HD), np.float32)
    for hh in range(HPC):
        for do in range(D):
            po = D * hh + do
            if do < D // 2:
                R[D * hh + do + D // 2, po] = -1.0
            else:
                R[D * hh + do - D // 2, po] = 1.0
    return R


def kernel(x, Wq, bq, Wk, bk, Wv, bv, Wo, bo):
    global LAST_RESULTS
    import ml_dtypes

    x = np.asarray(x, dtype=np.float32)
    Wq, bq = np.asarray(Wq, np.float32), np.asarray(bq, np.float32)
    Wk, bk = np.asarray(Wk, np.float32), np.asarray(bk, np.float32)
    Wv, bv = np.asarray(Wv, np.float32), np.asarray(bv, np.float32)
    Wo, bo = np.asarray(Wo, np.float32), np.asarray(bo, np.float32)

    dt_np = ml_dtypes.bfloat16
    T = B * S

    if "nc" not in _NC_CACHE:
        _NC_CACHE["nc"] = build_mha_nc()
    nc = _NC_CACHE["nc"]

    xT = np.ascontiguousarray(x.reshape(T, E).T).astype(dt_np)
    cos, sin = _rope_tables()
    cosS = np.tile(np.ascontiguousarray(cos.T), (HPC, 1)).astype(np.float32)
    sinS = np.tile(np.ascontiguousarray(sin.T), (HPC, 1)).astype(np.float32)
    R = _rot_matrix().astype(dt_np)

    in_maps = []
    for c in range(N_CORES):
        sl = slice(HD * c, HD * (c + 1))
        in_maps.append(
            {
                "xT": xT,
                "wq": np.ascontiguousarray(Wq[:, sl]).astype(dt_np),
                "wk": np.ascontiguousarray(Wk[:, sl]).astype(dt_np),
                "wv": np.ascontiguousarray(Wv[:, sl]).astype(dt_np),
                "bq": np.ascontiguousarray(bq[sl][:, None]).astype(np.float32),
                "bk": np.ascontiguousarray(bk[sl][:, None]).astype(np.float32),
                "bv": np.ascontiguousarray(bv[sl][:, None]).astype(np.float32),
                "wo": np.ascontiguousarray(Wo[sl, :]).astype(dt_np),
                "cosS": cosS,
                "sinS": sinS,
                "rot": R,
            }
        )

    res = bass_utils.run_bass_kernel_spmd(nc, in_maps, core_ids=list(range(N_CORES)))
    LAST_RESULTS = res

    out = np.zeros((T, E), np.float32)
    for c in range(N_CORES):
        out += res.results[c]["yp"].astype(np.float32)
    out += bo.astype(np.float32)
    return out.astype(np.float32).reshape(B, S, E)
